# revision 10
# baseline (speedup 1.0000x reference)
"""BCE-over-matched-pairs loss kernel for Trainium2 (8 NeuronCores).

Math: loss = sum_{k<K, b<B} bce(pred[b, r_k, c_k], gt[b, r_k, c_k]) / K
where bce(p, g) = -(g*max(log p, -100) + (1-g)*max(log1p(-p), -100)).

Host-side restructuring (index math only — all value math stays on
device): build the count matrix C[r, c] = |{k : (r_k, c_k) = (r, c)}|
via bincount.  Only ~10% of the 2048x2048 cells have C > 0, so instead
of streaming the full tensors we compact to the nonzero cells and
bucket them by count value v:

  - v == 1 and v == 2 buckets stream just (p, g); the constant count
    weight is baked into the PE reduction vectors.
    Per bucket: w * sum[g*A - g*B + B], A = log p, B = log(1-p).
  - v >= 3 cells stream (p, w*g, w*(1-g)) and accumulate
    sum [wg*A + we*B] directly.

Each core handles one batch b (8 batches, 8 cores) over all compacted
cells; identical shapes per core.

Engine split per core: ACT does the ln passes (plus a warmup instr so
the Ln table load overlaps the input DMA), DVE does only 2x-mode bf16
tensor_tensor multiplies, and the otherwise-idle PE does every
reduction as a [128,1]^T @ [128,F] matmul whose lhsT vector carries the
bucket weight (+-1, +-2), accumulating everything into one PSUM [1,512]
bank (column identity is irrelevant; the host sums the 512 lanes).

Accuracy: p, g are sent in bf16; p is clipped to 1 - 2^-8 (the largest
bf16 < 1) so log(1-p) never sees a bf16-rounded 1.0, and A uses a
2e-38 bias so p == 0 gives -86.8 instead of the reference's -100
clamp.  Measured ~1.7e-3 relative error on the final loss vs the 2e-2
gate.
"""

import numpy as np

B, N, M = 8, 2048, 2048
NCORES = 8
P = 128                      # SBUF partitions
MM = 512                     # PSUM bank width / matmul chunk
LOG_EPS = 2e-38              # log(p + eps): keeps p == 0 finite (-86.8)
PCLIP = np.float32(1.0 - 2 ** -8)   # largest bf16 strictly below 1.0
COL_PAD = 64                 # pad bucket column counts for cache stability

_NC_CACHE = {}


def _split_embedded_waits(nc, keep=1):
    """Hoist extra embedded semaphore waits into standalone EventSemaphore
    instructions.  This walrus build rejects instructions carrying more than
    ~1 wait + 1 update ("Too many sync wait commands"), but Tile emits
    multi-wait instructions; splitting is semantically identical since the
    engine sequencer executes the hoisted waits immediately before."""
    from concourse import mybir

    ctr = 0
    for fn in nc.m.functions:
        for blk in fn.blocks:
            new = []
            for inst in blk.instructions:
                si = inst.sync_info
                if si is not None and not isinstance(inst, mybir.InstEventSemaphore):
                    waits = list(si.on_wait or [])
                    ups = list(si.on_update or [])
                    if len(waits) > keep:
                        for w in waits[keep:]:
                            ctr += 1
                            es = mybir.InstEventSemaphore(name=f"hoistw-{ctr}")
                            es.engine = inst.engine
                            es.sync_info = mybir.SyncInfo(on_wait=[w], on_update=[])
                            new.append(es)
                        inst.sync_info = mybir.SyncInfo(
                            on_wait=waits[:keep], on_update=ups
                        )
                new.append(inst)
            blk.instructions = new


def _build_nc(f1, f2, fw, repeat=1):
    """Bucketed BCE kernel.  f1/f2 = column counts of the v=1 / v=2
    buckets, fw = column count of the weighted (v>=3) bucket; any may be
    0 to skip."""
    import concourse.bass as bass
    import concourse.tile as tile
    from concourse import mybir
    from contextlib import ExitStack

    nc = bass.Bass()
    bf16 = mybir.dt.bfloat16
    f32 = mybir.dt.float32
    Ln = mybir.ActivationFunctionType.Ln

    specs = []           # (tag, F, weighted, bucket_weight)
    if f1:
        specs.append(("b1", f1, False, 1.0))
    if f2:
        specs.append(("b2", f2, False, 2.0))
    if fw:
        specs.append(("bw", fw, True, 1.0))

    par = {}
    p_names, w_names = [], []
    for tag, F, weighted, _ in specs:
        par[tag + "_p"] = nc.declare_dram_parameter(tag + "_p", [P, F], bf16, isOutput=False)
        p_names.append(tag + "_p")
        if weighted:
            par[tag + "_wg"] = nc.declare_dram_parameter(tag + "_wg", [P, F], bf16, isOutput=False)
            par[tag + "_we"] = nc.declare_dram_parameter(tag + "_we", [P, F], bf16, isOutput=False)
            w_names += [tag + "_wg", tag + "_we"]
        else:
            par[tag + "_g"] = nc.declare_dram_parameter(tag + "_g", [P, F], bf16, isOutput=False)
            w_names.append(tag + "_g")

    out = nc.declare_dram_parameter("out", [1, 1], f32, isOutput=True)

    with tile.TileContext(nc) as tc, ExitStack() as ctx:
        io_pool = ctx.enter_context(tc.tile_pool(name="io", bufs=2))
        const_pool = ctx.enter_context(tc.tile_pool(name="const", bufs=1))
        psum_pool = ctx.enter_context(tc.tile_pool(name="psum", bufs=1, space="PSUM"))

        # Constant vectors: eps bias, PE weight vectors, PSUM zero-init rhs.
        eps_bias = const_pool.tile([P, 1], f32, tag="epsb")
        nc.vector.memset(eps_bias, LOG_EPS)
        warm = const_pool.tile([P, 1], bf16, tag="warm")
        # Warmup: loads the ACT Ln table while input DMAs are in flight.
        nc.scalar.activation(out=warm, in_=eps_bias, func=Ln, bias=1.0, scale=-1.0)

        wvecs = {}
        for wv in sorted({(w, s) for _, _, wd, w in specs
                          for s in ((1.0, -1.0) if not wd else (1.0,))}):
            w, s = wv
            t = const_pool.tile([P, 1], bf16, tag=f"wv{w}_{s}")
            nc.vector.memset(t, w * s)
            wvecs[wv] = t
        zvec = const_pool.tile([P, 1], bf16, tag="zv")
        nc.vector.memset(zvec, 0.0)
        zrhs = const_pool.tile([P, MM], bf16, tag="zr")
        nc.vector.memset(zrhs, 0.0)

        acc = psum_pool.tile([1, MM], f32)

        for rep in range(repeat):
            # All input DMAs up front: p streams first (ACT needs them
            # first), weight streams after.
            tiles = {}
            for name in p_names + w_names:
                t = io_pool.tile([P, par[name].shape[1]], bf16, tag=name)
                nc.sync.dma_start(out=t, in_=par[name][:, :])
                tiles[name] = t

            # PSUM zero-init matmul: lhsT is the zero vector, so the
            # (never-read) zrhs contents don't matter; start=True clears
            # the accumulation bank across its full width.
            nc.tensor.matmul(out=acc, lhsT=zvec, rhs=zrhs, start=(rep == 0), stop=False)

            mms = []     # deferred (wvec, tile) matmul operands

            # accb collects per-bucket sum(B) from the ACT accum port; one
            # tiny matmul per column folds it into PSUM with the bucket
            # weight.
            n_acc = sum(1 for _, _, wd, _ in specs if not wd)
            accb = io_pool.tile([P, max(n_acc, 1)], f32, tag="accb")
            bcol = 0

            for tag, F, weighted, w in specs:
                p_t = tiles[tag + "_p"]
                a_t = io_pool.tile([P, F], bf16, tag=tag + "_A")
                b_t = io_pool.tile([P, F], bf16, tag=tag + "_B")
                nc.scalar.activation(out=a_t, in_=p_t, func=Ln, bias=eps_bias, scale=1.0)
                if weighted:
                    nc.scalar.activation(out=b_t, in_=p_t, func=Ln, bias=1.0, scale=-1.0)
                    wg_t = tiles[tag + "_wg"]
                    we_t = tiles[tag + "_we"]
                    pa_t = io_pool.tile([P, F], bf16, tag=tag + "_pa")
                    pb_t = io_pool.tile([P, F], bf16, tag=tag + "_pb")
                    nc.vector.tensor_mul(pa_t, wg_t, a_t)    # wg * A
                    nc.vector.tensor_mul(pb_t, we_t, b_t)    # we * B
                    mms = [((w, 1.0), pa_t, None), ((w, 1.0), pb_t, None)]
                else:
                    # B pass accumulates sum(B) per partition for free.
                    nc.scalar.activation(out=b_t, in_=p_t, func=Ln, bias=1.0,
                                         scale=-1.0, accum_out=accb[:, bcol:bcol + 1])
                    g_t = tiles[tag + "_g"]
                    pa_t = io_pool.tile([P, F], bf16, tag=tag + "_pa")
                    pb_t = io_pool.tile([P, F], bf16, tag=tag + "_pb")
                    nc.vector.tensor_mul(pa_t, g_t, a_t)     # g * A
                    nc.vector.tensor_mul(pb_t, g_t, b_t)     # g * B
                    # + w*sum(g*A) - w*sum(g*B) + w*sum(B)
                    mms = [((w, 1.0), pa_t, None), ((w, -1.0), pb_t, None),
                           ((w, 1.0), accb, bcol)]
                    bcol += 1

                # Emit this bucket's reductions now so PE trails the DVE
                # products bucket by bucket.
                last_bucket = tag == specs[-1][0]
                for wv, t, col in mms:
                    is_last_mm = last_bucket and (wv, t, col) == mms[-1]
                    if col is not None:
                        # [P,1] f32 accumulator column -> one 1-wide matmul
                        nc.tensor.matmul(
                            out=acc[:, :1], lhsT=wvecs[wv],
                            rhs=t[:, col:col + 1],
                            start=False,
                            stop=(rep == repeat - 1 and is_last_mm),
                        )
                        continue
                    F_t = t.shape[1]
                    for j in range(0, F_t, MM):
                        cw = min(MM, F_t - j)
                        nc.tensor.matmul(
                            out=acc[:, :cw], lhsT=wvecs[wv],
                            rhs=t[:, j:j + cw],
                            start=False,
                            stop=(rep == repeat - 1 and is_last_mm
                                  and j + MM >= F_t),
                        )

        res = const_pool.tile([1, 1], f32, tag="res")
        nc.vector.tensor_reduce(
            out=res, in_=acc, axis=mybir.AxisListType.X, op=mybir.AluOpType.add)
        nc.sync.dma_start(out=out[:, :], in_=res)

    _split_embedded_waits(nc)
    return nc


def _get_nc(f1, f2, fw, repeat=1):
    key = (f1, f2, fw, repeat)
    if key not in _NC_CACHE:
        _NC_CACHE[key] = _build_nc(f1, f2, fw, repeat)
    return _NC_CACHE[key]


def _pad_cols(n):
    """Columns needed for n cells across P partitions, padded for cache
    key stability."""
    if n == 0:
        return 0
    f = -(-n // P)
    return -(-f // COL_PAD) * COL_PAD


def prepare_inputs(pred, gt, all_matches):
    """Host-side index restructuring: bincount, bucket by count value,
    gather per-batch values, pack bf16 [P, F] arrays (partition-major).
    Returns (in_maps, (f1, f2, fw))."""
    import ml_dtypes

    bf = ml_dtypes.bfloat16
    pred = np.asarray(pred, dtype=np.float32)
    gt = np.asarray(gt, dtype=np.float32)
    am = np.asarray(all_matches)

    idx = am[:, 0].astype(np.int64) * M + am[:, 1].astype(np.int64)
    c = np.bincount(idx, minlength=N * M)
    i1 = np.flatnonzero(c == 1)
    i2 = np.flatnonzero(c == 2)
    iw = np.flatnonzero(c >= 3)
    w = c[iw].astype(np.float32)
    f1, f2, fw = _pad_cols(i1.size), _pad_cols(i2.size), _pad_cols(iw.size)

    def pack(vals, F):
        out = np.zeros(P * F, dtype=bf)
        out[:vals.size] = vals.astype(bf)
        return out.reshape(P, F)

    pclip = bf(PCLIP)
    in_maps = []
    for b in range(B):
        pb = pred[b].ravel()
        gb = gt[b].ravel()
        m = {}
        if f1:
            m["b1_p"] = np.minimum(pack(pb[i1], f1), pclip)
            m["b1_g"] = pack(gb[i1], f1)
        if f2:
            m["b2_p"] = np.minimum(pack(pb[i2], f2), pclip)
            m["b2_g"] = pack(gb[i2], f2)
        if fw:
            gw = gb[iw]
            m["bw_p"] = np.minimum(pack(pb[iw], fw), pclip)
            m["bw_wg"] = pack(w * gw, fw)
            m["bw_we"] = pack(w * (1.0 - gw), fw)
        in_maps.append(m)
    return in_maps, (f1, f2, fw)


def kernel(pred_perm, gt_perm, all_matches):
    from concourse.bass_utils import run_bass_kernel_spmd

    am = np.asarray(all_matches)
    K = am.shape[0]
    in_maps, (f1, f2, fw) = prepare_inputs(pred_perm, gt_perm, all_matches)
    nc = _get_nc(f1, f2, fw)
    results = run_bass_kernel_spmd(nc, in_maps, list(range(NCORES))).results
    total = 0.0
    for r in results:
        total += float(np.sum(np.asarray(r["out"], dtype=np.float64)))
    return np.float32(-total / K)


# revision 13
# speedup vs baseline: 1.2562x; 1.2562x over previous
"""BCE-over-matched-pairs loss kernel for Trainium2 (8 NeuronCores).

Math: loss = sum_{k<K, b<B} bce(pred[b, r_k, c_k], gt[b, r_k, c_k]) / K
where bce(p, g) = -(g*max(log p, -100) + (1-g)*max(log1p(-p), -100)).

Host-side restructuring (index math only — all value math stays on
device): build the count matrix C[r, c] = |{k : (r_k, c_k) = (r, c)}|
via bincount.  Only ~10% of the 2048x2048 cells have C > 0, so instead
of streaming the full tensors we compact to the nonzero cells and
bucket them by count value v:

  - v == 1 and v == 2 buckets stream just (p, g); the constant count
    weight is baked into the PE reduction vectors.
    Per bucket: w * sum[g*A - g*B + B], A = log p, B = log(1-p).
  - v >= 3 cells stream (p, w*g, w*(1-g)) and accumulate
    sum [wg*A + we*B] directly.

Each core handles one batch b (8 batches, 8 cores) over all compacted
cells; identical shapes per core.

Engine split per core: ACT does the ln passes (plus a warmup instr so
the Ln table load overlaps the input DMA), DVE does only 2x-mode bf16
tensor_tensor multiplies, and the otherwise-idle PE does every
reduction as a [128,1]^T @ [128,F] matmul whose lhsT vector carries the
bucket weight (+-1, +-2), accumulating everything into one PSUM [1,512]
bank (column identity is irrelevant; the host sums the 512 lanes).

Accuracy: p, g are sent in bf16; p is clipped to 1 - 2^-8 (the largest
bf16 < 1) so log(1-p) never sees a bf16-rounded 1.0, and A uses a
2e-38 bias so p == 0 gives -86.8 instead of the reference's -100
clamp.  Measured ~1.7e-3 relative error on the final loss vs the 2e-2
gate.
"""

import numpy as np

B, N, M = 8, 2048, 2048
NCORES = 8
P = 128                      # SBUF partitions
MM = 512                     # PSUM bank width / matmul chunk
LOG_EPS = 2e-38              # log(p + eps): keeps p == 0 finite (-86.8)
PCLIP = np.float32(1.0 - 2 ** -8)   # largest bf16 strictly below 1.0
COL_PAD = 64                 # pad bucket column counts for cache stability

_NC_CACHE = {}


def _split_embedded_waits(nc, keep=1):
    """Hoist extra embedded semaphore waits into standalone EventSemaphore
    instructions.  This walrus build rejects instructions carrying more than
    ~1 wait + 1 update ("Too many sync wait commands"), but Tile emits
    multi-wait instructions; splitting is semantically identical since the
    engine sequencer executes the hoisted waits immediately before."""
    from concourse import mybir

    ctr = 0
    for fn in nc.m.functions:
        for blk in fn.blocks:
            new = []
            for inst in blk.instructions:
                si = inst.sync_info
                if si is not None and not isinstance(inst, mybir.InstEventSemaphore):
                    waits = list(si.on_wait or [])
                    ups = list(si.on_update or [])
                    if len(waits) > keep:
                        for w in waits[keep:]:
                            ctr += 1
                            es = mybir.InstEventSemaphore(name=f"hoistw-{ctr}")
                            es.engine = inst.engine
                            es.sync_info = mybir.SyncInfo(on_wait=[w], on_update=[])
                            new.append(es)
                        inst.sync_info = mybir.SyncInfo(
                            on_wait=waits[:keep], on_update=ups
                        )
                new.append(inst)
            blk.instructions = new


def _build_nc(f1, f2, fw, repeat=1):
    """Bucketed BCE kernel.  f1/f2 = column counts of the v=1 / v=2
    buckets, fw = column count of the weighted (v>=3) bucket; any may be
    0 to skip."""
    import concourse.bass as bass
    import concourse.tile as tile
    from concourse import mybir
    from contextlib import ExitStack

    nc = bass.Bass()
    bf16 = mybir.dt.bfloat16
    f32 = mybir.dt.float32
    Ln = mybir.ActivationFunctionType.Ln

    specs = []           # (tag, F, weighted, bucket_weight)
    if f1:
        specs.append(("b1", f1, False, 1.0))
    if f2:
        specs.append(("b2", f2, False, 2.0))
    if fw:
        specs.append(("bw", fw, True, 1.0))

    par = {}
    f2w = f2 + fw
    if f1:
        par["b1_p"] = nc.declare_dram_parameter("b1_p", [P, f1], bf16, isOutput=False)
        par["b1_g"] = nc.declare_dram_parameter("b1_g", [P, f1], bf16, isOutput=False)
    if f2:
        par["b2_p"] = nc.declare_dram_parameter("b2_p", [P, f2], bf16, isOutput=False)
        par["b2_g"] = nc.declare_dram_parameter("b2_g", [P, f2], bf16, isOutput=False)
    if fw:
        par["bw_p"] = nc.declare_dram_parameter("bw_p", [P, fw], bf16, isOutput=False)
        par["bw_wg"] = nc.declare_dram_parameter("bw_wg", [P, fw], bf16, isOutput=False)
        par["bw_we"] = nc.declare_dram_parameter("bw_we", [P, fw], bf16, isOutput=False)

    out = nc.declare_dram_parameter("out", [1, 2], f32, isOutput=True)

    with tile.TileContext(nc) as tc, ExitStack() as ctx:
        io_pool = ctx.enter_context(tc.tile_pool(name="io", bufs=2))
        const_pool = ctx.enter_context(tc.tile_pool(name="const", bufs=1))
        psum_pool = ctx.enter_context(tc.tile_pool(name="psum", bufs=1, space="PSUM"))

        eps_bias = const_pool.tile([P, 1], f32, tag="epsb")
        nc.vector.memset(eps_bias, LOG_EPS)
        warm = const_pool.tile([P, 1], bf16, tag="warm")
        # Warmup: loads the ACT Ln table while input DMAs are in flight.
        nc.scalar.activation(out=warm, in_=eps_bias, func=Ln, bias=1.0, scale=-1.0)

        wvecs = {}
        for w in (1.0, -1.0, 2.0, -2.0):
            t = const_pool.tile([P, 1], bf16, tag=f"wv{w}")
            nc.vector.memset(t, w)
            wvecs[w] = t
        wf2 = const_pool.tile([P, 1], f32, tag="wf2")
        nc.vector.memset(wf2, 1.0)
        zvec = const_pool.tile([P, 1], bf16, tag="zv")
        nc.vector.memset(zvec, 0.0)
        zrhs = const_pool.tile([P, MM], bf16, tag="zr")
        nc.vector.memset(zrhs, 0.0)

        acc = psum_pool.tile([1, MM], f32)
        acc2 = psum_pool.tile([1, MM], f32)
        use_acc2 = [False]

        for rep in range(repeat):
            st = rep == 0
            sp = rep == repeat - 1
            tiles = {}
            # p streams first (ACT is the critical chain), weights after.
            if f1:
                t = io_pool.tile([P, f1], bf16, tag="b1_p")
                nc.sync.dma_start(out=t, in_=par["b1_p"][:, :])
                tiles["b1_p"] = t
            if f2w:
                t = io_pool.tile([P, f2w], bf16, tag="p2w")
                if f2:
                    nc.sync.dma_start(out=t[:, :f2], in_=par["b2_p"][:, :])
                if fw:
                    nc.sync.dma_start(out=t[:, f2:], in_=par["bw_p"][:, :])
                tiles["p2w"] = t
            for name in ("b1_g", "b2_g", "bw_wg", "bw_we"):
                if name in par:
                    t = io_pool.tile([P, par[name].shape[1]], bf16, tag=name)
                    nc.sync.dma_start(out=t, in_=par[name][:, :])
                    tiles[name] = t

            # PSUM zero-init matmuls: lhsT is zero so zrhs contents are
            # irrelevant; start=True clears the full accumulation banks.
            nc.tensor.matmul(out=acc, lhsT=zvec, rhs=zrhs, start=st, stop=False)

            accb = io_pool.tile([P, 1], f32, tag="accb")

            # mm queue: (lhsT_weight, tile_or_slice, F, to_acc2_tail)
            prods = []

            if f1:
                p_t = tiles["b1_p"]
                a_t = io_pool.tile([P, f1], bf16, tag="b1_A")
                b_t = io_pool.tile([P, f1], bf16, tag="b1_B")
                nc.scalar.activation(out=a_t, in_=p_t, func=Ln, bias=eps_bias, scale=1.0)
                nc.scalar.activation(out=b_t, in_=p_t, func=Ln, bias=1.0,
                                     scale=-1.0, accum_out=accb)
                g_t = tiles["b1_g"]
                pa_t = io_pool.tile([P, f1], bf16, tag="b1_pa")
                pb_t = io_pool.tile([P, f1], bf16, tag="b1_pb")
                nc.vector.tensor_mul(pa_t, g_t, a_t)
                nc.vector.tensor_mul(pb_t, g_t, b_t)
                prods += [(1.0, pa_t, None), (-1.0, pb_t, None)]

            if f2w:
                p_t = tiles["p2w"]
                a_t = io_pool.tile([P, f2w], bf16, tag="2w_A")
                b_t = io_pool.tile([P, f2w], bf16, tag="2w_B")
                nc.scalar.activation(out=a_t, in_=p_t, func=Ln, bias=eps_bias, scale=1.0)
                nc.scalar.activation(out=b_t, in_=p_t, func=Ln, bias=1.0, scale=-1.0)
                if f2:
                    g_t = tiles["b2_g"]
                    pa_t = io_pool.tile([P, f2], bf16, tag="b2_pa")
                    pb_t = io_pool.tile([P, f2], bf16, tag="b2_pb")
                    nc.vector.tensor_mul(pa_t, g_t, a_t[:, :f2])
                    nc.vector.tensor_mul(pb_t, g_t, b_t[:, :f2])
                    # sum(B) for b2 via PE on the raw B slice, weight +2
                    prods += [(2.0, pa_t, None), (-2.0, pb_t, None),
                              (2.0, b_t, (0, f2))]
                if fw:
                    wg_t = tiles["bw_wg"]
                    we_t = tiles["bw_we"]
                    pa_t = io_pool.tile([P, fw], bf16, tag="bw_pa")
                    pb_t = io_pool.tile([P, fw], bf16, tag="bw_pb")
                    nc.vector.tensor_mul(pa_t, wg_t, a_t[:, f2:])
                    nc.vector.tensor_mul(pb_t, we_t, b_t[:, f2:])
                    prods += [(1.0, pa_t, None), (1.0, pb_t, None)]

            # b1's sum(B) accumulator column (f32): fold through PSUM.
            if f1:
                prods.append((None, accb, None))   # wf2 lhsT marker

            # The very last >=128-col chunk of the last product goes to
            # acc2 so the big acc reduce can overlap the tail.
            mmlist = []     # (wvec_tile, rhs_ap, width, which_acc)
            for wv, t, sl in prods:
                if wv is None:
                    mmlist.append((wf2, t[:, 0:1], 1, 0))
                    continue
                lo, hi = (sl if sl else (0, t.shape[1]))
                for j in range(lo, hi, MM):
                    cw = min(MM, hi - j)
                    mmlist.append((wvecs[wv], t[:, j:j + cw], cw, 0))
            # retarget the final chunk to acc2 when it's small enough to
            # make reduce2 cheap; otherwise split it.
            TAILW = 128
            lv, lap, lw, _ = mmlist[-1]
            if lw > TAILW:
                mmlist[-1] = (lv, lap[:, :lw - TAILW], lw - TAILW, 0)
                mmlist.append((lv, lap[:, lw - TAILW:], TAILW, 1))
            else:
                mmlist[-1] = (lv, lap, lw, 1)
            use_acc2[0] = True

            n_acc0 = sum(1 for m in mmlist if m[3] == 0)
            seen0 = seen1 = 0
            for wv_t, rhs_ap, cw, which in mmlist:
                if which == 0:
                    seen0 += 1
                    nc.tensor.matmul(out=acc[:, :cw], lhsT=wv_t, rhs=rhs_ap,
                                     start=False, stop=(sp and seen0 == n_acc0))
                else:
                    seen1 += 1
                    nc.tensor.matmul(out=acc2[:, :cw], lhsT=wv_t, rhs=rhs_ap,
                                     start=st and seen1 == 1,
                                     stop=(sp and seen1 == 1))

        res = const_pool.tile([1, 2], f32, tag="res")
        nc.vector.tensor_reduce(
            out=res[:, 0:1], in_=acc, axis=mybir.AxisListType.X, op=mybir.AluOpType.add)
        nc.vector.tensor_reduce(
            out=res[:, 1:2], in_=acc2[:, :TAILW], axis=mybir.AxisListType.X,
            op=mybir.AluOpType.add)
        nc.sync.dma_start(out=out[:, :], in_=res)

    _split_embedded_waits(nc)
    return nc


def _get_nc(f1, f2, fw, repeat=1):
    key = (f1, f2, fw, repeat)
    if key not in _NC_CACHE:
        _NC_CACHE[key] = _build_nc(f1, f2, fw, repeat)
    return _NC_CACHE[key]


def _pad_cols(n):
    """Columns needed for n cells across P partitions, padded for cache
    key stability."""
    if n == 0:
        return 0
    f = -(-n // P)
    return -(-f // COL_PAD) * COL_PAD


def prepare_inputs(pred, gt, all_matches):
    """Host-side index restructuring: bincount, bucket by count value,
    gather per-batch values, pack bf16 [P, F] arrays (partition-major).
    Returns (in_maps, (f1, f2, fw))."""
    import ml_dtypes

    bf = ml_dtypes.bfloat16
    pred = np.asarray(pred, dtype=np.float32)
    gt = np.asarray(gt, dtype=np.float32)
    am = np.asarray(all_matches)

    idx = am[:, 0].astype(np.int64) * M + am[:, 1].astype(np.int64)
    c = np.bincount(idx, minlength=N * M)
    i1 = np.flatnonzero(c == 1)
    i2 = np.flatnonzero(c == 2)
    iw = np.flatnonzero(c >= 3)
    w = c[iw].astype(np.float32)
    f1, f2, fw = _pad_cols(i1.size), _pad_cols(i2.size), _pad_cols(iw.size)

    def pack(vals, F):
        out = np.zeros(P * F, dtype=bf)
        out[:vals.size] = vals.astype(bf)
        return out.reshape(P, F)

    pclip = bf(PCLIP)
    in_maps = []
    for b in range(B):
        pb = pred[b].ravel()
        gb = gt[b].ravel()
        m = {}
        if f1:
            m["b1_p"] = np.minimum(pack(pb[i1], f1), pclip)
            m["b1_g"] = pack(gb[i1], f1)
        if f2:
            m["b2_p"] = np.minimum(pack(pb[i2], f2), pclip)
            m["b2_g"] = pack(gb[i2], f2)
        if fw:
            gw = gb[iw]
            m["bw_p"] = np.minimum(pack(pb[iw], fw), pclip)
            m["bw_wg"] = pack(w * gw, fw)
            m["bw_we"] = pack(w * (1.0 - gw), fw)
        in_maps.append(m)
    return in_maps, (f1, f2, fw)


def kernel(pred_perm, gt_perm, all_matches):
    from concourse.bass_utils import run_bass_kernel_spmd

    am = np.asarray(all_matches)
    K = am.shape[0]
    in_maps, (f1, f2, fw) = prepare_inputs(pred_perm, gt_perm, all_matches)
    nc = _get_nc(f1, f2, fw)
    results = run_bass_kernel_spmd(nc, in_maps, list(range(NCORES))).results
    total = 0.0
    for r in results:
        total += float(np.sum(np.asarray(r["out"], dtype=np.float64)))
    return np.float32(-total / K)
